# revision 1
# baseline (speedup 1.0000x reference)
"""Binary Jaccard index (IoU) kernel for Trainium2, 8 NeuronCores.

Reference computation (B=32, C=3, H=512, W=512, f32):
    a = (input >= 0.5), b = (target >= 0.5)
    inter[b,c] = sum_hw(a*b); union = sum(a) + sum(b) - inter
    iou = inter/union (1.0 where union == 0); return mean(iou)

Strategy: pure data parallel over the batch dim -- each of the 8 cores gets
4 batches = 12 (b,c) pairs, each pair a [128, 2048] f32 plane, processed in
4 chunks of [128, 512] for fine-grained DMA/compute overlap. Per chunk,
3 fused DVE ops produce the three per-partition partial sums directly:
  1. tensor_scalar(is_ge 0.5, accum add) : a-plane (bf16) + row-sums of a
  2. tensor_scalar(is_ge 0.5, accum add) : b-plane (bf16) + row-sums of b
  3. scalar_tensor_tensor(bypass, mult)  : a*b plane (bf16) + row-sums of a*b
Row-sums land in columns of a [128, 144] stats tile; one DMA writes it out.
The final partition/chunk-sums + IoU + mean over 96 pairs are a trivial
host-side epilogue (sums are integer-valued, exact in f32).
Cost-model time: 76.8us/core vs 73.4us pure-DMA floor (25.2 MB/core HBM).
"""

import numpy as np

import concourse.bacc as bacc
import concourse.bass as bass
import concourse.mybir as mybir
import concourse.tile as tile
from concourse.bass_utils import run_bass_kernel_spmd

N_CORES = 8
B, C, H, W = 32, 3, 512, 512
B_LOCAL = B // N_CORES          # 4 batches per core
PAIRS = B_LOCAL * C             # 12 (batch, channel) pairs per core
P = 128                         # SBUF partitions
F = (H * W) // P                # 2048 free-dim elements per pair
CHUNKS = 4                      # split each pair into chunks for finer overlap
FC = F // CHUNKS
THRESHOLD = 0.5

_CACHE = {}


def build_nc() -> bass.Bass:
    nc = bacc.Bacc("TRN2", target_bir_lowering=False, debug=False,
                   num_devices=N_CORES)
    x_d = nc.dram_tensor("x", [PAIRS, P, F], mybir.dt.float32,
                         kind="ExternalInput").ap()
    t_d = nc.dram_tensor("t", [PAIRS, P, F], mybir.dt.float32,
                         kind="ExternalInput").ap()
    s_d = nc.dram_tensor("stats", [P, PAIRS * CHUNKS * 3], mybir.dt.float32,
                         kind="ExternalOutput").ap()

    with tile.TileContext(nc) as tc:
        with tc.tile_pool(name="io", bufs=4) as io_pool, \
             tc.tile_pool(name="planes", bufs=2) as plane_pool, \
             tc.tile_pool(name="acc", bufs=1) as acc_pool:
            stats = acc_pool.tile([P, PAIRS * CHUNKS * 3], mybir.dt.float32)
            col = 0
            for i in range(PAIRS):
                for c in range(CHUNKS):
                    xt = io_pool.tile([P, FC], mybir.dt.float32, tag="x")
                    tt = io_pool.tile([P, FC], mybir.dt.float32, tag="t")
                    nc.sync.dma_start(out=xt, in_=x_d[i, :, c * FC:(c + 1) * FC])
                    nc.sync.dma_start(out=tt, in_=t_d[i, :, c * FC:(c + 1) * FC])
                    a = plane_pool.tile([P, FC], mybir.dt.bfloat16, tag="a")
                    b = plane_pool.tile([P, FC], mybir.dt.bfloat16, tag="b")
                    ab = plane_pool.tile([P, FC], mybir.dt.bfloat16, tag="ab")
                    nc.vector.tensor_scalar(
                        out=a, in0=xt, scalar1=THRESHOLD, scalar2=None,
                        op0=mybir.AluOpType.is_ge, op1=mybir.AluOpType.add,
                        accum_out=stats[:, col:col + 1])
                    nc.vector.tensor_scalar(
                        out=b, in0=tt, scalar1=THRESHOLD, scalar2=None,
                        op0=mybir.AluOpType.is_ge, op1=mybir.AluOpType.add,
                        accum_out=stats[:, col + 1:col + 2])
                    nc.vector.scalar_tensor_tensor(
                        out=ab, in0=a, scalar=1.0, in1=b,
                        op0=mybir.AluOpType.bypass, op1=mybir.AluOpType.mult,
                        accum_out=stats[:, col + 2:col + 3])
                    col += 3
            nc.sync.dma_start(out=s_d, in_=stats)
    nc.compile()
    return nc


def shard_inputs(input: np.ndarray, target: np.ndarray) -> list[dict]:
    in_maps = []
    for c in range(N_CORES):
        xs = input[c * B_LOCAL:(c + 1) * B_LOCAL].reshape(PAIRS, P, F)
        ts = target[c * B_LOCAL:(c + 1) * B_LOCAL].reshape(PAIRS, P, F)
        in_maps.append({"x": np.ascontiguousarray(xs),
                        "t": np.ascontiguousarray(ts)})
    return in_maps


def combine_outputs(stats_per_core: list[np.ndarray]) -> np.float32:
    ious = []
    for s in stats_per_core:
        # columns: [pair, chunk, quantity]; sum over partitions and chunks
        sums = s.astype(np.float64).sum(axis=0).reshape(PAIRS, CHUNKS, 3).sum(axis=1)
        sa, sb, sab = sums[:, 0], sums[:, 1], sums[:, 2]
        inter = sab
        union = sa + sb - sab
        iou = np.where(union > 0, inter / np.where(union > 0, union, 1.0), 1.0)
        ious.append(iou.astype(np.float32))
    return np.float32(np.mean(np.concatenate(ious)))


def kernel(input: np.ndarray, target: np.ndarray) -> np.ndarray:
    input = np.asarray(input, dtype=np.float32)
    target = np.asarray(target, dtype=np.float32)
    assert input.shape == (B, C, H, W) and target.shape == (B, C, H, W)

    if "nc" not in _CACHE:
        _CACHE["nc"] = build_nc()
    nc = _CACHE["nc"]

    res = run_bass_kernel_spmd(nc, shard_inputs(input, target),
                               core_ids=list(range(N_CORES)))
    return combine_outputs([r["stats"] for r in res.results])



# revision 2
# speedup vs baseline: 1.7058x; 1.7058x over previous
"""Binary Jaccard index (IoU) kernel for Trainium2, 8 NeuronCores.

Reference computation (B=32, C=3, H=512, W=512, f32):
    a = (input >= 0.5), b = (target >= 0.5)
    inter[b,c] = sum_hw(a*b); union = sum(a) + sum(b) - inter
    iou = inter/union (1.0 where union == 0); return mean(iou)

Strategy: pure data parallel over the batch dim -- each of the 8 cores gets
4 batches = 12 (b,c) pairs, each pair a [128, 2048] f32 plane in DRAM.

Device pipeline (per core):
  * Inputs stream in through gpsimd (SWDGE) casting DMAs f32 -> bf16, which
    halves DMA-engine occupancy (the kernel's roofline) to ~35us. bf16
    round-to-nearest moves only values within 2^-11 of 0.5 across the
    threshold (~0.05% of elements, one-sided), well inside tolerance.
  * Per pair i in 0..10 (three engines in parallel):
      DVE : a = (x >= 0.5)            [bf16 4x mode] + row-accum -> Sa
      Act : s = Sign(t - 0.49975586)  (+-1, never 0 for bf16 inputs)
                                      + row-accum -> 2*Sb - 2048
      DVE : c = a + s                 [bf16 2x mode]
      DVE : I = count(c >= 2)         [bf16 4x mode] + row-accum -> inter
  * Pair 11 is computed entirely on DVE (is_ge counts for both tensors) and
    split into a [0:1920] main part that streams FIRST and a tiny [1920:2048]
    tail whose two casting DMAs stream LAST, so the post-stream critical path
    is only sem-prop + ~0.4us of DVE + the stats DMA.
  * One [128, 39] f32 stats DMA out; host does the exact integer epilogue
    (per-pair IoU + mean over 96 pairs) in f64.

Cost-model time: ~41.3us/core vs ~35.1us casting-DMA floor
(76.8us for the previous f32-DMA version; 69.9us f32 DMA floor).
"""

import numpy as np

import concourse.bacc as bacc
import concourse.bass as bass
import concourse.mybir as mybir
import concourse.tile as tile
from concourse.bass_utils import run_bass_kernel_spmd

N_CORES = 8
B, C, H, W = 32, 3, 512, 512
B_LOCAL = B // N_CORES          # 4 batches per core
PAIRS = B_LOCAL * C             # 12 (batch, channel) pairs per core
P = 128                         # SBUF partitions
F = (H * W) // P                # 2048 free-dim elements per pair
THRESHOLD = 0.5
# No bf16 value equals this f32 constant, so Sign(t + (-BIAS_ABS)) is never 0,
# and t >= 0.5  <=>  Sign(t - BIAS_ABS) == +1 for bf16 t.
BIAS = -(0.5 - 2.0 ** -12)

N_ACT = PAIRS - 1               # pairs 0..10 use the Activation engine for Sb
F_MAIN = 1920                   # pair 11 main slice
F_TAIL = F - F_MAIN             # pair 11 tail slice (last DMAs in the stream)
NCOLS = 3 * N_ACT + 6           # stats columns

_CACHE = {}


def build_nc() -> bass.Bass:
    nc = bacc.Bacc("TRN2", target_bir_lowering=False, debug=False,
                   num_devices=N_CORES)
    x_d = nc.dram_tensor("x", [PAIRS, P, F], mybir.dt.float32,
                         kind="ExternalInput").ap()
    t_d = nc.dram_tensor("t", [PAIRS, P, F], mybir.dt.float32,
                         kind="ExternalInput").ap()
    s_d = nc.dram_tensor("stats", [P, NCOLS], mybir.dt.float32,
                         kind="ExternalOutput").ap()

    ts_kw = dict(scalar2=None, op0=mybir.AluOpType.is_ge,
                 op1=mybir.AluOpType.add)

    with tile.TileContext(nc) as tc:
        with tc.tile_pool(name="io", bufs=4) as io_pool, \
             tc.tile_pool(name="p12", bufs=1) as p12_pool, \
             tc.tile_pool(name="acc", bufs=1) as acc_pool:
            stats = acc_pool.tile([P, NCOLS], mybir.dt.float32)
            bias_t = acc_pool.tile([P, 1], mybir.dt.float32)
            nc.vector.memset(bias_t, BIAS)

            # ---- DMA stream (gpsimd casting DMAs, in execution order) ----
            # pair 11 main first: its 2.8us DVE chain runs early.
            xm = p12_pool.tile([P, F_MAIN], mybir.dt.bfloat16)
            tm = p12_pool.tile([P, F_MAIN], mybir.dt.bfloat16)
            nc.gpsimd.dma_start(out=xm, in_=x_d[N_ACT, :, 0:F_MAIN])
            nc.gpsimd.dma_start(out=tm, in_=t_d[N_ACT, :, 0:F_MAIN])
            # pairs 0..10 (Act consumes t, so give Act the earliest data).
            xb, tb = [], []
            for i in range(N_ACT):
                xi = io_pool.tile([P, F], mybir.dt.bfloat16, tag="x")
                ti = io_pool.tile([P, F], mybir.dt.bfloat16, tag="t")
                nc.gpsimd.dma_start(out=xi, in_=x_d[i])
                nc.gpsimd.dma_start(out=ti, in_=t_d[i])
                xb.append(xi)
                tb.append(ti)
            # pair 11 tail last: shortest possible post-stream chain.
            xt = p12_pool.tile([P, F_TAIL], mybir.dt.bfloat16)
            tt = p12_pool.tile([P, F_TAIL], mybir.dt.bfloat16)
            nc.gpsimd.dma_start(out=tt, in_=t_d[N_ACT, :, F_MAIN:F])
            nc.gpsimd.dma_start(out=xt, in_=x_d[N_ACT, :, F_MAIN:F])

            # ---- pair 11 main on DVE (early) ----
            am = p12_pool.tile([P, F_MAIN], mybir.dt.bfloat16)
            bm = p12_pool.tile([P, F_MAIN], mybir.dt.bfloat16)
            cm = p12_pool.tile([P, F_MAIN], mybir.dt.bfloat16)
            c0 = 3 * N_ACT
            nc.vector.tensor_scalar(out=am, in0=xm, scalar1=THRESHOLD,
                                    accum_out=stats[:, c0:c0 + 1], **ts_kw)
            nc.vector.tensor_scalar(out=bm, in0=tm, scalar1=THRESHOLD,
                                    accum_out=stats[:, c0 + 1:c0 + 2], **ts_kw)
            nc.vector.tensor_tensor(out=cm, in0=am, in1=bm,
                                    op=mybir.AluOpType.add)
            nc.vector.tensor_scalar(out=cm, in0=cm, scalar1=2.0,
                                    accum_out=stats[:, c0 + 2:c0 + 3], **ts_kw)

            # ---- pairs 0..10: DVE + Act ----
            for i in range(N_ACT):
                ai = io_pool.tile([P, F], mybir.dt.bfloat16, tag="a")
                si = io_pool.tile([P, F], mybir.dt.bfloat16, tag="s")
                ci = io_pool.tile([P, F], mybir.dt.bfloat16, tag="c")
                nc.vector.tensor_scalar(out=ai, in0=xb[i], scalar1=THRESHOLD,
                                        accum_out=stats[:, 3 * i:3 * i + 1],
                                        **ts_kw)
                nc.scalar.activation(out=si, in_=tb[i],
                                     func=mybir.ActivationFunctionType.Sign,
                                     bias=bias_t,
                                     accum_out=stats[:, 3 * i + 1:3 * i + 2])
                nc.vector.tensor_tensor(out=ci, in0=ai, in1=si,
                                        op=mybir.AluOpType.add)
                nc.vector.tensor_scalar(out=ci, in0=ci, scalar1=2.0,
                                        accum_out=stats[:, 3 * i + 2:3 * i + 3],
                                        **ts_kw)

            # ---- pair 11 tail on DVE (the post-stream critical path) ----
            at = p12_pool.tile([P, F_TAIL], mybir.dt.bfloat16)
            bt = p12_pool.tile([P, F_TAIL], mybir.dt.bfloat16)
            ct = p12_pool.tile([P, F_TAIL], mybir.dt.bfloat16)
            nc.vector.tensor_scalar(out=bt, in0=tt, scalar1=THRESHOLD,
                                    accum_out=stats[:, c0 + 4:c0 + 5], **ts_kw)
            nc.vector.tensor_scalar(out=at, in0=xt, scalar1=THRESHOLD,
                                    accum_out=stats[:, c0 + 3:c0 + 4], **ts_kw)
            nc.vector.tensor_tensor(out=ct, in0=at, in1=bt,
                                    op=mybir.AluOpType.add)
            nc.vector.tensor_scalar(out=ct, in0=ct, scalar1=2.0,
                                    accum_out=stats[:, c0 + 5:c0 + 6], **ts_kw)

            nc.sync.dma_start(out=s_d, in_=stats)
    nc.compile()
    return nc


def shard_inputs(input: np.ndarray, target: np.ndarray) -> list[dict]:
    in_maps = []
    for c in range(N_CORES):
        xs = input[c * B_LOCAL:(c + 1) * B_LOCAL].reshape(PAIRS, P, F)
        ts = target[c * B_LOCAL:(c + 1) * B_LOCAL].reshape(PAIRS, P, F)
        in_maps.append({"x": np.ascontiguousarray(xs),
                        "t": np.ascontiguousarray(ts)})
    return in_maps


def combine_outputs(stats_per_core: list[np.ndarray]) -> np.float32:
    ious = []
    for s in stats_per_core:
        cols = s.astype(np.float64).sum(axis=0)          # [NCOLS]
        sa = np.empty(PAIRS)
        sb = np.empty(PAIRS)
        it = np.empty(PAIRS)
        for i in range(N_ACT):
            sa[i] = cols[3 * i]
            sb[i] = (cols[3 * i + 1] + P * F) / 2.0      # accum = 2*Sb - P*F
            it[i] = cols[3 * i + 2]
        c0 = 3 * N_ACT
        sa[N_ACT] = cols[c0] + cols[c0 + 3]
        sb[N_ACT] = cols[c0 + 1] + cols[c0 + 4]
        it[N_ACT] = cols[c0 + 2] + cols[c0 + 5]
        union = sa + sb - it
        iou = np.where(union > 0, it / np.where(union > 0, union, 1.0), 1.0)
        ious.append(iou.astype(np.float32))
    return np.float32(np.mean(np.concatenate(ious)))


def kernel(input: np.ndarray, target: np.ndarray) -> np.ndarray:
    input = np.asarray(input, dtype=np.float32)
    target = np.asarray(target, dtype=np.float32)
    assert input.shape == (B, C, H, W) and target.shape == (B, C, H, W)

    if "nc" not in _CACHE:
        _CACHE["nc"] = build_nc()
    nc = _CACHE["nc"]

    res = run_bass_kernel_spmd(nc, shard_inputs(input, target),
                               core_ids=list(range(N_CORES)))
    return combine_outputs([r["stats"] for r in res.results])


# revision 10
# speedup vs baseline: 1.7608x; 1.0323x over previous
"""Binary Jaccard index (IoU) kernel for Trainium2, 8 NeuronCores.

Reference computation (B=32, C=3, H=512, W=512, f32):
    a = (input >= 0.5), b = (target >= 0.5)
    inter[b,c] = sum_hw(a*b); union = sum(a) + sum(b) - inter
    iou = inter/union (1.0 where union == 0); return mean(iou)

Strategy: pure data parallel over the batch dim -- each of the 8 cores gets
4 batches = 12 (b,c) pairs, each pair a [128, 2048] f32 plane in DRAM.

Device pipeline (per core):
  * Inputs stream in through gpsimd (SWDGE) casting DMAs f32 -> bf16, which
    halves DMA-engine occupancy (the kernel's roofline) to ~35us. bf16
    round-to-nearest only moves values within 2^-11 of 0.5 across the
    threshold (~0.05% of elements, one-sided), far inside tolerance.
  * Every chunk of work uses the Activation engine for the x-side count and
    DVE for the rest, with each chunk's x DMA issued ahead of the previous
    chunk's t DMA so Act's ~3.2us latency hides under the t transfer:
      Act : s = Sign(x - 0.49975586)  (+-1, never 0 in bf16) + row-accum
      DVE : b = (t >= 0.5)  [4x bf16 mode] + row-accum -> Sb
      DVE : c = s + b       [2x]
      DVE : I = count(c >= 2) [4x] + row-accum -> inter
  * Pairs 0..8 are whole-pair chunks; pairs 9..11 are split into 1024-elem
    half chunks so the post-stream critical path shrinks. The 900ns
    DMA-completion semaphore plus the final chunk's DVE chain floor the
    overhang at ~2.2us past the last transfer.
  * One [128, 45] f32 stats DMA out; host does the exact integer epilogue
    (per-pair IoU + mean over 96 pairs) in f64.
"""

import numpy as np

import concourse.bacc as bacc
import concourse.bass as bass
import concourse.mybir as mybir
import concourse.tile as tile
from concourse.bass_utils import run_bass_kernel_spmd

N_CORES = 8
B, C, H, W = 32, 3, 512, 512
B_LOCAL = B // N_CORES          # 4 batches per core
PAIRS = B_LOCAL * C             # 12 (batch, channel) pairs per core
P = 128                         # SBUF partitions
F = (H * W) // P                # 2048 free-dim elements per pair
THRESHOLD = 0.5
# No bf16 value equals this f32 constant, so Sign(x - BIAS_ABS) is never 0,
# and x >= 0.5  <=>  Sign(x - BIAS_ABS) == +1 for bf16 x.
BIAS = -(0.5 - 2.0 ** -12)

# Work chunks (pair, start, size), in stream order. Whole pairs first, the
# last three pairs split in halves to shorten the post-stream DVE chain.
CHUNKS = [(i, 0, F) for i in range(9)]
for i in (9, 10, 11):
    CHUNKS += [(i, 0, 1024), (i, 1024, 1024)]
NCH = len(CHUNKS)
NCOLS = 3 * NCH

_CACHE = {}


def build_nc() -> bass.Bass:
    nc = bacc.Bacc("TRN2", target_bir_lowering=False, debug=False,
                   num_devices=N_CORES)
    x_d = nc.dram_tensor("x", [PAIRS, P, F], mybir.dt.float32,
                         kind="ExternalInput").ap()
    t_d = nc.dram_tensor("t", [PAIRS, P, F], mybir.dt.float32,
                         kind="ExternalInput").ap()
    s_d = nc.dram_tensor("stats", [P, NCOLS], mybir.dt.float32,
                         kind="ExternalOutput").ap()

    ts_kw = dict(scalar2=None, op0=mybir.AluOpType.is_ge,
                 op1=mybir.AluOpType.add)

    def x_ap(c):
        pi, st, sz = CHUNKS[c]
        return x_d[pi, :, st:st + sz]

    def t_ap(c):
        pi, st, sz = CHUNKS[c]
        return t_d[pi, :, st:st + sz]

    with tile.TileContext(nc) as tc:
        with tc.tile_pool(name="io", bufs=6) as io_pool, \
             tc.tile_pool(name="tail", bufs=1) as tail_pool, \
             tc.tile_pool(name="acc", bufs=1) as acc_pool:
            stats = acc_pool.tile([P, NCOLS], mybir.dt.float32)
            bias_t = acc_pool.tile([P, 1], mybir.dt.float32)
            nc.vector.memset(bias_t, BIAS)

            def mk(c, which, dt=mybir.dt.bfloat16):
                pi, st, sz = CHUNKS[c]
                if sz == F:
                    return io_pool.tile([P, sz], dt, tag=which,
                                        name=f"{which}{c}")
                return tail_pool.tile([P, sz], dt, name=f"{which}{c}")

            # ---- DMA stream: x one chunk ahead of t ----
            xb = [mk(c, "x") for c in range(NCH)]
            tb = [mk(c, "t") for c in range(NCH)]
            nc.gpsimd.dma_start(out=xb[0], in_=x_ap(0))
            for c in range(NCH):
                if c + 1 < NCH:
                    nc.gpsimd.dma_start(out=xb[c + 1], in_=x_ap(c + 1))
                nc.gpsimd.dma_start(out=tb[c], in_=t_ap(c))

            # ---- Act engine: x-side Sign counts ----
            sb = [mk(c, "s") for c in range(NCH)]
            for c in range(NCH):
                nc.scalar.activation(out=sb[c], in_=xb[c],
                                     func=mybir.ActivationFunctionType.Sign,
                                     bias=bias_t,
                                     accum_out=stats[:, 3 * c:3 * c + 1])


            # ---- DVE: t-side count + min-combine, in stream order ----
            # inter = count(min(x,t) >= 0.5): no dependency on the Act pass.
            for c in range(NCH):
                pi, st, sz = CHUNKS[c]
                bi = mk(c, "b")
                ci = mk(c, "c")
                nc.vector.tensor_scalar(out=bi, in0=tb[c], scalar1=THRESHOLD,
                                        accum_out=stats[:, 3 * c + 1:3 * c + 2],
                                        **ts_kw)
                nc.vector.tensor_tensor(out=ci, in0=xb[c], in1=tb[c],
                                        op=mybir.AluOpType.min)
                nc.vector.tensor_scalar(out=ci, in0=ci, scalar1=THRESHOLD,
                                        accum_out=stats[:, 3 * c + 2:3 * c + 3],
                                        **ts_kw)

            nc.sync.dma_start(out=s_d, in_=stats)
    nc.compile()
    return nc


def shard_inputs(input: np.ndarray, target: np.ndarray) -> list[dict]:
    in_maps = []
    for c in range(N_CORES):
        xs = input[c * B_LOCAL:(c + 1) * B_LOCAL].reshape(PAIRS, P, F)
        ts = target[c * B_LOCAL:(c + 1) * B_LOCAL].reshape(PAIRS, P, F)
        in_maps.append({"x": np.ascontiguousarray(xs),
                        "t": np.ascontiguousarray(ts)})
    return in_maps


def combine_outputs(stats_per_core: list[np.ndarray]) -> np.float32:
    ious = []
    for s in stats_per_core:
        cols = s.astype(np.float64).sum(axis=0)          # [NCOLS]
        sa = np.zeros(PAIRS)
        sb = np.zeros(PAIRS)
        it = np.zeros(PAIRS)
        for c, (pi, st, sz) in enumerate(CHUNKS):
            sa[pi] += (cols[3 * c] + P * sz) / 2.0       # accum = 2*Sa - P*sz
            sb[pi] += cols[3 * c + 1]
            it[pi] += cols[3 * c + 2]
        union = sa + sb - it
        iou = np.where(union > 0, it / np.where(union > 0, union, 1.0), 1.0)
        ious.append(iou.astype(np.float32))
    return np.float32(np.mean(np.concatenate(ious)))


def kernel(input: np.ndarray, target: np.ndarray) -> np.ndarray:
    input = np.asarray(input, dtype=np.float32)
    target = np.asarray(target, dtype=np.float32)
    assert input.shape == (B, C, H, W) and target.shape == (B, C, H, W)

    if "nc" not in _CACHE:
        _CACHE["nc"] = build_nc()
    nc = _CACHE["nc"]

    res = run_bass_kernel_spmd(nc, shard_inputs(input, target),
                               core_ids=list(range(N_CORES)))
    return combine_outputs([r["stats"] for r in res.results])


# revision 21
# speedup vs baseline: 1.7627x; 1.0011x over previous
"""Binary Jaccard index (IoU) kernel for Trainium2, 8 NeuronCores.

Reference computation (B=32, C=3, H=512, W=512, f32):
    a = (input >= 0.5), b = (target >= 0.5)
    inter[b,c] = sum_hw(a*b); union = sum(a) + sum(b) - inter
    iou = inter/union (1.0 where union == 0); return mean(iou)

Strategy: pure data parallel over the batch dim -- each of the 8 cores gets
4 batches = 12 (b,c) pairs, each pair a [128, 2048] f32 plane in DRAM.

Device pipeline (per core):
  * Inputs stream in through gpsimd (SWDGE) casting DMAs f32 -> bf16, which
    halves DMA-engine occupancy (the kernel's roofline) to ~35us. bf16
    round-to-nearest only moves values within 2^-11 of 0.5 across the
    threshold (~0.05% of elements, one-sided), far inside tolerance.
  * Every chunk of work uses the Activation engine for the x-side count and
    DVE for the rest, with each chunk's x DMA issued ahead of the previous
    chunk's t DMA so Act's ~3.2us latency hides under the t transfer:
      Act : s = Sign(x - 0.49975586)  (+-1, never 0 in bf16) + row-accum
      DVE : b = (t >= 0.5)  [4x bf16 mode] + row-accum -> Sb
      DVE : c = s + b       [2x]
      DVE : I = count(c >= 2) [4x] + row-accum -> inter
  * Pairs 0..8 are whole-pair chunks; pairs 9..11 are split into 1024-elem
    half chunks so the post-stream critical path shrinks. The 900ns
    DMA-completion semaphore plus the final chunk's DVE chain floor the
    overhang at ~2.2us past the last transfer.
  * One [128, 45] f32 stats DMA out; host does the exact integer epilogue
    (per-pair IoU + mean over 96 pairs) in f64.
"""

import numpy as np

import concourse.bacc as bacc
import concourse.bass as bass
import concourse.mybir as mybir
import concourse.tile as tile
from concourse.bass_utils import run_bass_kernel_spmd

N_CORES = 8
B, C, H, W = 32, 3, 512, 512
B_LOCAL = B // N_CORES          # 4 batches per core
PAIRS = B_LOCAL * C             # 12 (batch, channel) pairs per core
P = 128                         # SBUF partitions
F = (H * W) // P                # 2048 free-dim elements per pair
THRESHOLD = 0.5
# No bf16 value equals this f32 constant, so Sign(x - BIAS_ABS) is never 0,
# and x >= 0.5  <=>  Sign(x - BIAS_ABS) == +1 for bf16 x.
BIAS = -(0.5 - 2.0 ** -12)

# Work chunks (pair, start, size), in stream order. Whole pairs first, the
# last three pairs split in halves to shorten the post-stream DVE chain.
CHUNKS = [(i, 0, F) for i in range(9)]
for i in (9, 10, 11):
    CHUNKS += [(i, 0, 1280), (i, 1280, 768)]
NCH = len(CHUNKS)
NCOLS = 3 * NCH
# Chunks whose Sb (t-side count) runs on the Activation engine (as a Sign
# accum) instead of DVE -- Act has slack and nothing downstream consumes it.
ACT_SB = {6, 7}
DVE_SA = set()
POOL_CHUNKS = set()  # gpsimd compute breaks the PJRT lowering; keep off

_CACHE = {}


def build_nc() -> bass.Bass:
    nc = bacc.Bacc("TRN2", target_bir_lowering=False, debug=False,
                   num_devices=N_CORES)
    x_d = nc.dram_tensor("x", [PAIRS, P, F], mybir.dt.float32,
                         kind="ExternalInput").ap()
    t_d = nc.dram_tensor("t", [PAIRS, P, F], mybir.dt.float32,
                         kind="ExternalInput").ap()
    s_d = nc.dram_tensor("stats", [P, NCOLS], mybir.dt.float32,
                         kind="ExternalOutput").ap()

    ts_kw = dict(scalar2=None, op0=mybir.AluOpType.is_ge,
                 op1=mybir.AluOpType.add)

    def x_ap(c):
        pi, st, sz = CHUNKS[c]
        return x_d[pi, :, st:st + sz]

    def t_ap(c):
        pi, st, sz = CHUNKS[c]
        return t_d[pi, :, st:st + sz]

    with tile.TileContext(nc) as tc:
        with tc.tile_pool(name="io", bufs=6) as io_pool, \
             tc.tile_pool(name="tail", bufs=1) as tail_pool, \
             tc.tile_pool(name="acc", bufs=1) as acc_pool:
            stats = acc_pool.tile([P, NCOLS], mybir.dt.float32)
            bias_t = acc_pool.tile([P, 1], mybir.dt.float32)
            nc.vector.memset(bias_t, BIAS)

            def mk(c, which, dt=mybir.dt.bfloat16):
                pi, st, sz = CHUNKS[c]
                if sz == F:
                    return io_pool.tile([P, sz], dt, tag=which,
                                        name=f"{which}{c}")
                return tail_pool.tile([P, sz], dt, name=f"{which}{c}")

            # ---- DMA stream: x one chunk ahead of t ----
            xb = [mk(c, "x") for c in range(NCH)]
            tb = [mk(c, "t") for c in range(NCH)]
            nc.gpsimd.dma_start(out=xb[0], in_=x_ap(0))
            for c in range(NCH):
                if c + 1 < NCH:
                    nc.gpsimd.dma_start(out=xb[c + 1], in_=x_ap(c + 1))
                nc.gpsimd.dma_start(out=tb[c], in_=t_ap(c))

            # ---- Act engine: x-side Sign counts (+ t-side for ACT_SB) ----
            sb = [mk(c, "s") for c in range(NCH)]
            st_ = {c: mk(c, "u") for c in ACT_SB}
            for c in range(NCH):
                if c not in DVE_SA:
                    nc.scalar.activation(out=sb[c], in_=xb[c],
                                         func=mybir.ActivationFunctionType.Sign,
                                         bias=bias_t,
                                         accum_out=stats[:, 3 * c:3 * c + 1])
                if c in ACT_SB:
                    nc.scalar.activation(out=st_[c], in_=tb[c],
                                         func=mybir.ActivationFunctionType.Sign,
                                         bias=bias_t,
                                         accum_out=stats[:, 3 * c + 1:3 * c + 2])


            # ---- DVE: t-side count + min-combine, in stream order ----
            # inter = count(min(x,t) >= 0.5): no dependency on the Act pass.
            for c in range(NCH):
                pi, st, sz = CHUNKS[c]
                eng = nc.gpsimd if c in POOL_CHUNKS else nc.vector
                ci = mk(c, "c")
                if c in DVE_SA:
                    ai = mk(c, "b")
                    nc.vector.tensor_scalar(
                        out=ai, in0=xb[c], scalar1=THRESHOLD,
                        accum_out=stats[:, 3 * c:3 * c + 1], **ts_kw)
                if c not in ACT_SB:
                    bi = mk(c, "b")
                    eng.tensor_scalar(
                        out=bi, in0=tb[c], scalar1=THRESHOLD,
                        accum_out=stats[:, 3 * c + 1:3 * c + 2], **ts_kw)
                eng.tensor_tensor(out=ci, in0=xb[c], in1=tb[c],
                                  op=mybir.AluOpType.min)
                eng.tensor_scalar(out=ci, in0=ci, scalar1=THRESHOLD,
                                  accum_out=stats[:, 3 * c + 2:3 * c + 3],
                                  **ts_kw)

            nc.sync.dma_start(out=s_d, in_=stats)
    nc.compile()
    return nc


def shard_inputs(input: np.ndarray, target: np.ndarray) -> list[dict]:
    in_maps = []
    for c in range(N_CORES):
        xs = input[c * B_LOCAL:(c + 1) * B_LOCAL].reshape(PAIRS, P, F)
        ts = target[c * B_LOCAL:(c + 1) * B_LOCAL].reshape(PAIRS, P, F)
        in_maps.append({"x": np.ascontiguousarray(xs),
                        "t": np.ascontiguousarray(ts)})
    return in_maps


def combine_outputs(stats_per_core: list[np.ndarray]) -> np.float32:
    ious = []
    for s in stats_per_core:
        cols = s.astype(np.float64).sum(axis=0)          # [NCOLS]
        sa = np.zeros(PAIRS)
        sb = np.zeros(PAIRS)
        it = np.zeros(PAIRS)
        for c, (pi, st, sz) in enumerate(CHUNKS):
            if c in DVE_SA:
                sa[pi] += cols[3 * c]                    # plain is_ge count
            else:
                sa[pi] += (cols[3 * c] + P * sz) / 2.0   # accum = 2*Sa - P*sz
            if c in ACT_SB:
                sb[pi] += (cols[3 * c + 1] + P * sz) / 2.0
            else:
                sb[pi] += cols[3 * c + 1]
            it[pi] += cols[3 * c + 2]
        union = sa + sb - it
        iou = np.where(union > 0, it / np.where(union > 0, union, 1.0), 1.0)
        ious.append(iou.astype(np.float32))
    return np.float32(np.mean(np.concatenate(ious)))


def kernel(input: np.ndarray, target: np.ndarray) -> np.ndarray:
    input = np.asarray(input, dtype=np.float32)
    target = np.asarray(target, dtype=np.float32)
    assert input.shape == (B, C, H, W) and target.shape == (B, C, H, W)

    if "nc" not in _CACHE:
        _CACHE["nc"] = build_nc()
    nc = _CACHE["nc"]

    res = run_bass_kernel_spmd(nc, shard_inputs(input, target),
                               core_ids=list(range(N_CORES)))
    return combine_outputs([r["stats"] for r in res.results])


# revision 25
# speedup vs baseline: 1.7656x; 1.0017x over previous
"""Binary Jaccard index (IoU) kernel for Trainium2, 8 NeuronCores.

Reference computation (B=32, C=3, H=512, W=512, f32):
    a = (input >= 0.5), b = (target >= 0.5)
    inter[b,c] = sum_hw(a*b); union = sum(a) + sum(b) - inter
    iou = inter/union (1.0 where union == 0); return mean(iou)

Strategy: pure data parallel over the batch dim -- each of the 8 cores gets
4 batches = 12 (b,c) pairs, each pair a [128, 2048] f32 plane in DRAM.

Device pipeline (per core):
  * Inputs stream in through gpsimd (SWDGE) casting DMAs f32 -> bf16, which
    halves DMA-engine occupancy (the kernel's roofline) to ~35us. bf16
    round-to-nearest only moves values within 2^-11 of 0.5 across the
    threshold (~0.05% of elements, one-sided), far inside tolerance.
  * Every chunk of work uses the Activation engine for the x-side count and
    DVE for the rest, with each chunk's x DMA issued ahead of the previous
    chunk's t DMA so Act's ~3.2us latency hides under the t transfer:
      Act : s = Sign(x - 0.49975586)  (+-1, never 0 in bf16) + row-accum
      DVE : b = (t >= 0.5)  [4x bf16 mode] + row-accum -> Sb
      DVE : c = s + b       [2x]
      DVE : I = count(c >= 2) [4x] + row-accum -> inter
  * Pairs 0..8 are whole-pair chunks; pairs 9..11 are split into 1024-elem
    half chunks so the post-stream critical path shrinks. The 900ns
    DMA-completion semaphore plus the final chunk's DVE chain floor the
    overhang at ~2.2us past the last transfer.
  * One [128, 45] f32 stats DMA out; host does the exact integer epilogue
    (per-pair IoU + mean over 96 pairs) in f64.
"""

import numpy as np

import concourse.bacc as bacc
import concourse.bass as bass
import concourse.mybir as mybir
import concourse.tile as tile
from concourse.bass_utils import run_bass_kernel_spmd

N_CORES = 8
B, C, H, W = 32, 3, 512, 512
B_LOCAL = B // N_CORES          # 4 batches per core
PAIRS = B_LOCAL * C             # 12 (batch, channel) pairs per core
P = 128                         # SBUF partitions
F = (H * W) // P                # 2048 free-dim elements per pair
THRESHOLD = 0.5
# No bf16 value equals this f32 constant, so Sign(x - BIAS_ABS) is never 0,
# and x >= 0.5  <=>  Sign(x - BIAS_ABS) == +1 for bf16 x.
BIAS = -(0.5 - 2.0 ** -12)

# Work chunks (pair, start, size), in stream order. Whole pairs first, the
# last three pairs split in halves to shorten the post-stream DVE chain.
CHUNKS = [(i, 0, F) for i in range(9)]
for i in (9, 10, 11):
    CHUNKS += [(i, 0, 1280), (i, 1280, 768)]
NCH = len(CHUNKS)
NCOLS = 3 * NCH
# Chunks whose Sb (t-side count) runs on the Activation engine (as a Sign
# accum) instead of DVE -- Act has slack and nothing downstream consumes it.
ACT_SB = {6, 7}
DVE_SA = set()
POOL_CHUNKS = set()  # gpsimd compute breaks the PJRT lowering; keep off

_CACHE = {}


def build_nc() -> bass.Bass:
    nc = bacc.Bacc("TRN2", target_bir_lowering=False, debug=False,
                   num_devices=N_CORES)
    x_d = nc.dram_tensor("x", [PAIRS, P, F], mybir.dt.float32,
                         kind="ExternalInput").ap()
    t_d = nc.dram_tensor("t", [PAIRS, P, F], mybir.dt.float32,
                         kind="ExternalInput").ap()
    s_d = nc.dram_tensor("stats", [P, NCOLS], mybir.dt.float32,
                         kind="ExternalOutput").ap()

    ts_kw = dict(scalar2=None, op0=mybir.AluOpType.is_ge,
                 op1=mybir.AluOpType.add)

    def x_ap(c):
        pi, st, sz = CHUNKS[c]
        return x_d[pi, :, st:st + sz]

    def t_ap(c):
        pi, st, sz = CHUNKS[c]
        return t_d[pi, :, st:st + sz]

    with tile.TileContext(nc) as tc:
        with tc.tile_pool(name="io", bufs=6) as io_pool, \
             tc.tile_pool(name="tail", bufs=1) as tail_pool, \
             tc.tile_pool(name="acc", bufs=1) as acc_pool:
            stats = acc_pool.tile([P, NCOLS], mybir.dt.float32)
            bias_t = acc_pool.tile([P, 1], mybir.dt.float32)
            nc.vector.memset(bias_t, BIAS)

            def mk(c, which, dt=mybir.dt.bfloat16):
                pi, st, sz = CHUNKS[c]
                if sz == F:
                    return io_pool.tile([P, sz], dt, tag=which,
                                        name=f"{which}{c}")
                return tail_pool.tile([P, sz], dt, name=f"{which}{c}")

            # ---- DMA stream: x one chunk ahead of t ----
            xb = [mk(c, "x") for c in range(NCH)]
            tb = [mk(c, "t") for c in range(NCH)]
            nc.gpsimd.dma_start(out=xb[0], in_=x_ap(0))
            for c in range(NCH):
                if c + 1 < NCH:
                    nc.gpsimd.dma_start(out=xb[c + 1], in_=x_ap(c + 1))
                nc.gpsimd.dma_start(out=tb[c], in_=t_ap(c))

            # ---- Act engine: x-side Sign counts (+ t-side for ACT_SB) ----
            sb = [mk(c, "s") for c in range(NCH)]
            st_ = {c: tail_pool.tile([P, CHUNKS[c][2]], mybir.dt.bfloat16,
                         name=f"u{c}") for c in ACT_SB}
            for c in range(NCH):
                if c not in DVE_SA:
                    nc.scalar.activation(out=sb[c], in_=xb[c],
                                         func=mybir.ActivationFunctionType.Sign,
                                         bias=bias_t,
                                         accum_out=stats[:, 3 * c:3 * c + 1])
                if c in ACT_SB:
                    # interleaved: lands in Act's early idle windows
                    nc.scalar.activation(out=st_[c], in_=tb[c],
                                         func=mybir.ActivationFunctionType.Sign,
                                         bias=bias_t,
                                         accum_out=stats[:, 3 * c + 1:3 * c + 2])


            # ---- DVE: t-side count + min-combine, in stream order ----
            # inter = count(min(x,t) >= 0.5): no dependency on the Act pass.
            for c in range(NCH):
                pi, st, sz = CHUNKS[c]
                eng = nc.gpsimd if c in POOL_CHUNKS else nc.vector
                ci = mk(c, "c")
                if c in DVE_SA:
                    ai = tail_pool.tile([P, sz], mybir.dt.bfloat16,
                                        name=f"a{c}")
                    nc.vector.tensor_scalar(
                        out=ai, in0=xb[c], scalar1=THRESHOLD,
                        accum_out=stats[:, 3 * c:3 * c + 1], **ts_kw)
                if c not in ACT_SB:
                    bi = mk(c, "b")
                    eng.tensor_scalar(
                        out=bi, in0=tb[c], scalar1=THRESHOLD,
                        accum_out=stats[:, 3 * c + 1:3 * c + 2], **ts_kw)
                eng.tensor_tensor(out=ci, in0=xb[c], in1=tb[c],
                                  op=mybir.AluOpType.min)
                eng.tensor_scalar(out=ci, in0=ci, scalar1=THRESHOLD,
                                  accum_out=stats[:, 3 * c + 2:3 * c + 3],
                                  **ts_kw)

            # split stats: bulk goes out as soon as chunks 0..12 finish
            # (DMA engines are idle by then); only the last two chunks' 6
            # columns ride the final dependency chain (56ns transfer).
            nc.sync.dma_start(out=s_d[:, :3 * (NCH - 2)],
                              in_=stats[:, :3 * (NCH - 2)])
            nc.sync.dma_start(out=s_d[:, 3 * (NCH - 2):],
                              in_=stats[:, 3 * (NCH - 2):])
    nc.compile()
    return nc


def shard_inputs(input: np.ndarray, target: np.ndarray) -> list[dict]:
    in_maps = []
    for c in range(N_CORES):
        xs = input[c * B_LOCAL:(c + 1) * B_LOCAL].reshape(PAIRS, P, F)
        ts = target[c * B_LOCAL:(c + 1) * B_LOCAL].reshape(PAIRS, P, F)
        in_maps.append({"x": np.ascontiguousarray(xs),
                        "t": np.ascontiguousarray(ts)})
    return in_maps


def combine_outputs(stats_per_core: list[np.ndarray]) -> np.float32:
    ious = []
    for s in stats_per_core:
        cols = s.astype(np.float64).sum(axis=0)          # [NCOLS]
        sa = np.zeros(PAIRS)
        sb = np.zeros(PAIRS)
        it = np.zeros(PAIRS)
        for c, (pi, st, sz) in enumerate(CHUNKS):
            if c in DVE_SA:
                sa[pi] += cols[3 * c]                    # plain is_ge count
            else:
                sa[pi] += (cols[3 * c] + P * sz) / 2.0   # accum = 2*Sa - P*sz
            if c in ACT_SB:
                sb[pi] += (cols[3 * c + 1] + P * sz) / 2.0
            else:
                sb[pi] += cols[3 * c + 1]
            it[pi] += cols[3 * c + 2]
        union = sa + sb - it
        iou = np.where(union > 0, it / np.where(union > 0, union, 1.0), 1.0)
        ious.append(iou.astype(np.float32))
    return np.float32(np.mean(np.concatenate(ious)))


def kernel(input: np.ndarray, target: np.ndarray) -> np.ndarray:
    input = np.asarray(input, dtype=np.float32)
    target = np.asarray(target, dtype=np.float32)
    assert input.shape == (B, C, H, W) and target.shape == (B, C, H, W)

    if "nc" not in _CACHE:
        _CACHE["nc"] = build_nc()
    nc = _CACHE["nc"]

    res = run_bass_kernel_spmd(nc, shard_inputs(input, target),
                               core_ids=list(range(N_CORES)))
    return combine_outputs([r["stats"] for r in res.results])


# revision 28
# speedup vs baseline: 1.7762x; 1.0060x over previous
"""Binary Jaccard index (IoU) kernel for Trainium2, 8 NeuronCores.

Reference computation (B=32, C=3, H=512, W=512, f32):
    a = (input >= 0.5), b = (target >= 0.5)
    inter[b,c] = sum_hw(a*b); union = sum(a) + sum(b) - inter
    iou = inter/union (1.0 where union == 0); return mean(iou)

Strategy: pure data parallel over the batch dim -- each of the 8 cores gets
4 batches = 12 (b,c) pairs, each pair a [128, 2048] f32 plane in DRAM.

Device pipeline (per core):
  * Inputs stream in through gpsimd (SWDGE) casting DMAs f32 -> bf16, which
    halves DMA-engine occupancy (the kernel's roofline) to ~35us. bf16
    round-to-nearest only moves values within 2^-11 of 0.5 across the
    threshold (~0.05% of elements, one-sided), far inside tolerance.
  * Every chunk of work uses the Activation engine for the x-side count and
    DVE for the rest, with each chunk's x DMA issued ahead of the previous
    chunk's t DMA so Act's ~3.2us latency hides under the t transfer:
      Act : s = Sign(x - 0.49975586)  (+-1, never 0 in bf16) + row-accum
      DVE : b = (t >= 0.5)  [4x bf16 mode] + row-accum -> Sb
      DVE : c = s + b       [2x]
      DVE : I = count(c >= 2) [4x] + row-accum -> inter
  * Pairs 0..8 are whole-pair chunks; pairs 9..11 are split into 1024-elem
    half chunks so the post-stream critical path shrinks. The 900ns
    DMA-completion semaphore plus the final chunk's DVE chain floor the
    overhang at ~2.2us past the last transfer.
  * One [128, 45] f32 stats DMA out; host does the exact integer epilogue
    (per-pair IoU + mean over 96 pairs) in f64.
"""

import numpy as np

import concourse.bacc as bacc
import concourse.bass as bass
import concourse.mybir as mybir
import concourse.tile as tile
from concourse.bass_utils import run_bass_kernel_spmd

N_CORES = 8
B, C, H, W = 32, 3, 512, 512
B_LOCAL = B // N_CORES          # 4 batches per core
PAIRS = B_LOCAL * C             # 12 (batch, channel) pairs per core
P = 128                         # SBUF partitions
F = (H * W) // P                # 2048 free-dim elements per pair
THRESHOLD = 0.5
# No bf16 value equals this f32 constant, so Sign(x - BIAS_ABS) is never 0,
# and x >= 0.5  <=>  Sign(x - BIAS_ABS) == +1 for bf16 x.
BIAS = -(0.5 - 2.0 ** -12)

# Work chunks (pair, start, size), in stream order. Whole pairs first, the
# last three pairs split in halves to shorten the post-stream DVE chain.
CHUNKS = [(i, 0, F) for i in range(9)]
for i in (9, 10, 11):
    CHUNKS += [(i, 0, 1280), (i, 1280, 768)]
NCH = len(CHUNKS)
NCOLS = 3 * NCH
# Chunks whose Sb (t-side count) runs on the Activation engine (as a Sign
# accum) instead of DVE -- Act has slack and nothing downstream consumes it.
ACT_SB = {6, 7, 12}
DVE_SA = set()
POOL_CHUNKS = set()  # gpsimd compute breaks the PJRT lowering; keep off

_CACHE = {}


def build_nc() -> bass.Bass:
    nc = bacc.Bacc("TRN2", target_bir_lowering=False, debug=False,
                   num_devices=N_CORES)
    x_d = nc.dram_tensor("x", [PAIRS, P, F], mybir.dt.float32,
                         kind="ExternalInput").ap()
    t_d = nc.dram_tensor("t", [PAIRS, P, F], mybir.dt.float32,
                         kind="ExternalInput").ap()
    s_d = nc.dram_tensor("stats", [P, NCOLS], mybir.dt.float32,
                         kind="ExternalOutput").ap()

    ts_kw = dict(scalar2=None, op0=mybir.AluOpType.is_ge,
                 op1=mybir.AluOpType.add)

    def x_ap(c):
        pi, st, sz = CHUNKS[c]
        return x_d[pi, :, st:st + sz]

    def t_ap(c):
        pi, st, sz = CHUNKS[c]
        return t_d[pi, :, st:st + sz]

    with tile.TileContext(nc) as tc:
        with tc.tile_pool(name="io", bufs=6) as io_pool, \
             tc.tile_pool(name="tail", bufs=1) as tail_pool, \
             tc.tile_pool(name="acc", bufs=1) as acc_pool:
            stats = acc_pool.tile([P, NCOLS], mybir.dt.float32)
            bias_t = acc_pool.tile([P, 1], mybir.dt.float32)
            nc.vector.memset(bias_t, BIAS)

            def mk(c, which, dt=mybir.dt.bfloat16):
                pi, st, sz = CHUNKS[c]
                if sz == F:
                    return io_pool.tile([P, sz], dt, tag=which,
                                        name=f"{which}{c}")
                return tail_pool.tile([P, sz], dt, name=f"{which}{c}")

            # ---- DMA stream: x one chunk ahead of t ----
            xb = [mk(c, "x") for c in range(NCH)]
            tb = [mk(c, "t") for c in range(NCH)]
            nc.gpsimd.dma_start(out=xb[0], in_=x_ap(0))
            for c in range(NCH):
                if c + 1 < NCH:
                    nc.gpsimd.dma_start(out=xb[c + 1], in_=x_ap(c + 1))
                nc.gpsimd.dma_start(out=tb[c], in_=t_ap(c))

            # ---- Act engine: x-side Sign counts (+ t-side for ACT_SB) ----
            sb = [mk(c, "s") for c in range(NCH)]
            st_ = {c: tail_pool.tile([P, CHUNKS[c][2]], mybir.dt.bfloat16,
                         name=f"u{c}") for c in ACT_SB}
            for c in range(NCH):
                if c not in DVE_SA:
                    nc.scalar.activation(out=sb[c], in_=xb[c],
                                         func=mybir.ActivationFunctionType.Sign,
                                         bias=bias_t,
                                         accum_out=stats[:, 3 * c:3 * c + 1])
                if c in ACT_SB:
                    # interleaved: lands in Act's early idle windows
                    nc.scalar.activation(out=st_[c], in_=tb[c],
                                         func=mybir.ActivationFunctionType.Sign,
                                         bias=bias_t,
                                         accum_out=stats[:, 3 * c + 1:3 * c + 2])


            # ---- DVE: t-side count + min-combine, in stream order ----
            # inter = count(min(x,t) >= 0.5): no dependency on the Act pass.
            for c in range(NCH):
                pi, st, sz = CHUNKS[c]
                eng = nc.gpsimd if c in POOL_CHUNKS else nc.vector
                ci = mk(c, "c")
                if c in DVE_SA:
                    ai = tail_pool.tile([P, sz], mybir.dt.bfloat16,
                                        name=f"a{c}")
                    nc.vector.tensor_scalar(
                        out=ai, in0=xb[c], scalar1=THRESHOLD,
                        accum_out=stats[:, 3 * c:3 * c + 1], **ts_kw)
                if c not in ACT_SB:
                    bi = mk(c, "b")
                    eng.tensor_scalar(
                        out=bi, in0=tb[c], scalar1=THRESHOLD,
                        accum_out=stats[:, 3 * c + 1:3 * c + 2], **ts_kw)
                eng.tensor_tensor(out=ci, in0=xb[c], in1=tb[c],
                                  op=mybir.AluOpType.min)
                eng.tensor_scalar(out=ci, in0=ci, scalar1=THRESHOLD,
                                  accum_out=stats[:, 3 * c + 2:3 * c + 3],
                                  **ts_kw)

            # split stats: bulk goes out as soon as chunks 0..12 finish
            # (DMA engines are idle by then); only the last two chunks' 6
            # columns ride the final dependency chain (56ns transfer).
            nc.sync.dma_start(out=s_d[:, :3 * (NCH - 2)],
                              in_=stats[:, :3 * (NCH - 2)])
            nc.sync.dma_start(out=s_d[:, 3 * (NCH - 2):],
                              in_=stats[:, 3 * (NCH - 2):])
    nc.compile()
    return nc


def shard_inputs(input: np.ndarray, target: np.ndarray) -> list[dict]:
    in_maps = []
    for c in range(N_CORES):
        xs = input[c * B_LOCAL:(c + 1) * B_LOCAL].reshape(PAIRS, P, F)
        ts = target[c * B_LOCAL:(c + 1) * B_LOCAL].reshape(PAIRS, P, F)
        in_maps.append({"x": np.ascontiguousarray(xs),
                        "t": np.ascontiguousarray(ts)})
    return in_maps


def combine_outputs(stats_per_core: list[np.ndarray]) -> np.float32:
    ious = []
    for s in stats_per_core:
        cols = s.astype(np.float64).sum(axis=0)          # [NCOLS]
        sa = np.zeros(PAIRS)
        sb = np.zeros(PAIRS)
        it = np.zeros(PAIRS)
        for c, (pi, st, sz) in enumerate(CHUNKS):
            if c in DVE_SA:
                sa[pi] += cols[3 * c]                    # plain is_ge count
            else:
                sa[pi] += (cols[3 * c] + P * sz) / 2.0   # accum = 2*Sa - P*sz
            if c in ACT_SB:
                sb[pi] += (cols[3 * c + 1] + P * sz) / 2.0
            else:
                sb[pi] += cols[3 * c + 1]
            it[pi] += cols[3 * c + 2]
        union = sa + sb - it
        iou = np.where(union > 0, it / np.where(union > 0, union, 1.0), 1.0)
        ious.append(iou.astype(np.float32))
    return np.float32(np.mean(np.concatenate(ious)))


def kernel(input: np.ndarray, target: np.ndarray) -> np.ndarray:
    input = np.asarray(input, dtype=np.float32)
    target = np.asarray(target, dtype=np.float32)
    assert input.shape == (B, C, H, W) and target.shape == (B, C, H, W)

    if "nc" not in _CACHE:
        _CACHE["nc"] = build_nc()
    nc = _CACHE["nc"]

    res = run_bass_kernel_spmd(nc, shard_inputs(input, target),
                               core_ids=list(range(N_CORES)))
    return combine_outputs([r["stats"] for r in res.results])
